# revision 1
# baseline (speedup 1.0000x reference)
"""Causal self-attention kernel for Trainium2, 8-way sharded.

Problem: B=2, T=2048, C=1024, NH=16, hd=64. fp32 in/out.

Sharding: core = (batch b, head-group g of 4 heads). Each core computes its
4 heads' attention for its batch plus the partial output projection
y_local @ Wo[g*256:(g+1)*256, :]; the host sums the 4 partials per batch
(biases bv/bo are folded in exactly via a host-side correction row).

v2 schedule (vs baseline):
  - All matmul tensors bf16 (x, weights, q/k, P, V, y): halves DMA both
    ways, enables FWL weight loads. Scores/projections accumulate fp32 in
    PSUM so only operand quantization is lost (~0.5%, budget is 2e-2).
  - x is DMAed T-slice-major (slice = 512 tq x all 8 C-chunks = 0.5MB)
    so projections stream as slices land; PE is busy from ~4us and HAM
    stays warm instead of idling 24us for the full-x DMA.
  - Window-major over head-pairs: for each tq window w, both pairs' score/
    exp/PV run back-to-back, so the ScalarE exp stream never starves
    (baseline had a 42us ScalarE hole between pairs).
  - Per-window normalization: denominators for the 4 (pair,head) lanes of
    window w are batched into one [128,512] DVE reciprocal (lanes at
    partitions 0/32/64/96); the norm + output projection for window w then
    run as PE filler inside window w+1's (ACT-bound) attention instead of
    a 64us serialized tail. The last window is normalized and projected in
    tq halves so recip/norm/out/DMA pipeline, with the final DMAs split
    across three queues.
  - Output DMAed as bf16 partials; host sums in float64.
"""
import contextlib

import ml_dtypes
import numpy as np

import concourse.bass as bass
import concourse.tile as tile
from concourse import bacc, mybir
from concourse import bass_utils

bass_utils.upload_artifacts = lambda tmpdir: "local://skipped"

B, T, C = 2, 2048, 1024
NH, HD = 16, 64
NHL = 4            # heads per core
CLOC = NHL * HD    # 256 local channels
NCH = C // 128     # 8 contraction chunks
TQW = 512          # tq window / T-slice width
NW = T // TQW      # 4 windows
NTT = T // 128     # 16 t-tiles / tk-chunks
VSTR = HD + 8      # 72: v cols per head + 8 ones cols (denom at row 64)
LAG = 2            # PV trails S^T by this many chunk-groups
F32R = mybir.dt.float32r
F32 = mybir.dt.float32
BF16 = mybir.dt.bfloat16

_cache = {}


def _build():
    nc = bacc.Bacc("TRN2", target_bir_lowering=False, debug=False, num_devices=8)

    # x slice-major: slice s holds chunks c at offset (s*NCH+c)*TQW
    xt_ap = nc.dram_tensor("xt", [128, NW * NCH * TQW], BF16, kind="ExternalInput").ap()
    wq_ap = nc.dram_tensor("wq", [128, 2 * NCH * 128], BF16, kind="ExternalInput").ap()
    wk_ap = nc.dram_tensor("wk", [128, 2 * NCH * 128], BF16, kind="ExternalInput").ap()
    wv_ap = nc.dram_tensor("wv", [128, NCH * CLOC], BF16, kind="ExternalInput").ap()
    wo_ap = nc.dram_tensor("wo", [128, 2 * C], BF16, kind="ExternalInput").ap()
    bq_ap = nc.dram_tensor("bq", [2, 128, 1], F32, kind="ExternalInput").ap()
    bk_ap = nc.dram_tensor("bk", [2, 128, 1], F32, kind="ExternalInput").ap()
    ones_ap = nc.dram_tensor("ones", [128, NTT, NHL, 8], BF16, kind="ExternalInput").ap()
    selw_ap = nc.dram_tensor("selw", [128, 2 * 128], F32, kind="ExternalInput").ap()
    tri_ap = nc.dram_tensor("tri", [128, 128], BF16, kind="ExternalInput").ap()
    out_ap = nc.dram_tensor("out", [T, C], BF16, kind="ExternalOutput").ap()

    with tile.TileContext(nc) as tc, contextlib.ExitStack() as ctx:
        sb = ctx.enter_context(tc.tile_pool(name="sb", bufs=1))
        ost_pool = ctx.enter_context(tc.tile_pool(name="ost", bufs=3))
        pt_pool = ctx.enter_context(tc.tile_pool(name="ptp", bufs=10))
        ps = ctx.enter_context(tc.tile_pool(name="ps", bufs=1, space="PSUM"))

        # ---- persistent SBUF tensors ----
        wqs = sb.tile([128, 2 * NCH * 128], BF16, tag="wqs")
        wks = sb.tile([128, 2 * NCH * 128], BF16, tag="wks")
        wvs = sb.tile([128, NCH * CLOC], BF16, tag="wvs")
        wos = sb.tile([128, 2 * C], BF16, tag="wos")
        xts = sb.tile([128, NW * NCH * TQW], BF16, tag="xts")
        qts = [sb.tile([128, T], BF16, tag=f"qt{p}", name=f"qt{p}") for p in range(2)]
        kts = [sb.tile([128, T], BF16, tag=f"kt{p}", name=f"kt{p}") for p in range(2)]
        vna = sb.tile([128, NTT * NHL * VSTR], BF16, tag="vna")
        yts = [sb.tile([128, T], BF16, tag=f"yt{p}", name=f"yt{p}") for p in range(2)]
        bqs = [sb.tile([128, 1], F32, tag=f"bq{p}", name=f"bqs{p}") for p in range(2)]
        bks = [sb.tile([128, 1], F32, tag=f"bk{p}", name=f"bks{p}") for p in range(2)]
        selw = sb.tile([128, 2 * 128], F32, tag="selw")
        dsb = sb.tile([128, TQW], F32, tag="dsb")
        nc.vector.memset(dsb[:], 1.0)
        tri = sb.tile([128, 128], BF16, tag="tri")
        rcoll = sb.tile([128, TQW], F32, tag="rcoll")

        # ---- input DMAs ----
        # scalar queue: tiny constants + w weights (all land < ~5us, before
        # the exp stream owns ScalarE). sync: x slices 0,1. gpsimd: wv/ones
        # then x slices 2,3 and wo.
        for p in range(2):
            nc.scalar.dma_start(bqs[p][:], bq_ap[p])
            nc.scalar.dma_start(bks[p][:], bk_ap[p])
        HW = NCH * 128  # 1024 cols = one pair's weight block
        nc.scalar.dma_start(wqs[:, 0:HW], wq_ap[:, 0:HW])
        nc.scalar.dma_start(wks[:, 0:HW], wk_ap[:, 0:HW])
        nc.scalar.dma_start(wqs[:, HW:2 * HW], wq_ap[:, HW:2 * HW])
        nc.scalar.dma_start(wks[:, HW:2 * HW], wk_ap[:, HW:2 * HW])
        nc.scalar.dma_start(selw[:], selw_ap[:])
        nc.scalar.dma_start(tri[:], tri_ap[:])
        nc.sync.dma_start(xts[:, 0:NCH * TQW], xt_ap[:, 0:NCH * TQW])
        nc.sync.dma_start(xts[:, NCH * TQW:2 * NCH * TQW],
                          xt_ap[:, NCH * TQW:2 * NCH * TQW])
        nc.sync.dma_start(xts[:, 2 * NCH * TQW:3 * NCH * TQW],
                          xt_ap[:, 2 * NCH * TQW:3 * NCH * TQW])
        nc.gpsimd.dma_start(wvs[:], wv_ap[:])
        vna4 = vna[:].rearrange("p (t h v) -> p t h v", t=NTT, h=NHL)
        nc.gpsimd.dma_start(vna4[:, :, :, HD:HD + 8], ones_ap[:])
        nc.gpsimd.dma_start(xts[:, 3 * NCH * TQW:4 * NCH * TQW],
                            xt_ap[:, 3 * NCH * TQW:4 * NCH * TQW])
        nc.gpsimd.dma_start(wos[:], wo_ap[:])

        pt_tiles = {}

        # ---------- emission primitives ----------
        def warm():
            wtile = sb.tile([128, 640], BF16, tag="warm")
            wjunk = sb.tile([128, 8], F32, tag="wjunk")
            nc.vector.memset(wtile[:], 0.0)
            wp = ps.tile([128, TQW], F32, tag="proj", bufs=2, name="warm_ps")
            for i in range(14):
                nc.tensor.matmul(wp[:], wtile[:, 0:128], wtile[:, 128:640],
                                 start=True, stop=True)
            nc.vector.tensor_copy(wjunk[:], wp[:, 0:8])
            # load the exp table while DMAs stream
            wact = sb.tile([128, 8], BF16, tag="wact")
            nc.scalar.activation(wact[:], wp[:, 0:8],
                                 mybir.ActivationFunctionType.Exp, scale=0.125)

        def qk_window(p, ty, w):
            wsb, dst, bias = ((wqs, qts[p], bqs[p]), (wks, kts[p], bks[p]))[ty]
            acc = ps.tile([128, TQW], F32, tag="proj", bufs=2, name=f"qk{p}{ty}{w}")
            for c in range(NCH):
                nc.tensor.matmul(
                    acc[:], wsb[:, (p * NCH + c) * 128:(p * NCH + c + 1) * 128],
                    xts[:, (w * NCH + c) * TQW:(w * NCH + c + 1) * TQW],
                    start=(c == 0), stop=(c == NCH - 1))
            nc.vector.tensor_scalar_add(dst[:, w * TQW:(w + 1) * TQW],
                                        acc[:], bias[:])

        def v_tile(tt):
            s, q = tt // 4, tt % 4
            acc = ps.tile([128, CLOC], F32, tag="proj", bufs=2, name=f"v{tt}")
            for c in range(NCH):
                base = (s * NCH + c) * TQW + q * 128
                nc.tensor.matmul(acc[:], xts[:, base:base + 128],
                                 wvs[:, c * CLOC:(c + 1) * CLOC],
                                 start=(c == 0), stop=(c == NCH - 1))
            base = tt * NHL * VSTR
            dst = vna[:, base:base + NHL * VSTR].rearrange("p (h d) -> p h d", h=NHL)
            nc.vector.tensor_copy(dst[:, :, 0:HD],
                                  acc[:].rearrange("p (h d) -> p h d", h=NHL))

        def st_slot(p, w, g, h):
            qt, kt = qts[p], kts[p]
            nchunks = 4 * (w + 1)
            c0 = 2 * g
            st = ps.tile([128, 1024], F32, tag="st", bufs=2,
                         name=f"st{p}{w}{g}{h}")
            for j in range(2):
                c = c0 + j
                nc.tensor.matmul(
                    st[:, j * TQW:(j + 1) * TQW],
                    kt[h * 64:(h + 1) * 64, c * 128:(c + 1) * 128],
                    qt[h * 64:(h + 1) * 64, w * TQW:(w + 1) * TQW],
                    start=True, stop=True)
            pt = pt_pool.tile([128, 1024], BF16, tag="pt", name=f"pt{p}{w}{g}{h}")
            nc.scalar.activation(pt[:], st[:], mybir.ActivationFunctionType.Exp,
                                 scale=0.125)
            if c0 + 1 >= nchunks - 4:
                # causal mask: diag 128-block at cols [128*jp, 128*jp+128),
                # left of it = 0
                for j in range(2):
                    jp = (c0 + j) - 4 * w
                    if jp > 0:
                        nc.gpsimd.memset(pt[:, j * TQW: j * TQW + 128 * jp], 0.0)
                    dslc = pt[:, j * TQW + 128 * jp: j * TQW + 128 * jp + 128]
                    nc.vector.tensor_mul(dslc, dslc, tri[:])
            pt_tiles[(p, w, g, h)] = pt

        def pv_group(p, w, g, h, accs):
            nchunks = 4 * (w + 1)
            c0 = 2 * g
            pt = pt_tiles.pop((p, w, g, h))
            for j in range(2):
                c = c0 + j
                vbase = c * NHL * VSTR + (2 * p + h) * VSTR
                nc.tensor.matmul(
                    accs[h][0:VSTR, :],
                    vna[:, vbase:vbase + VSTR],
                    pt[:, j * TQW:(j + 1) * TQW],
                    start=(c0 == 0 and j == 0),
                    stop=(c0 == nchunks - 2 and j == 1))

        def pv_tail(p, w, h, accs):
            # y (unnormalized) to SBUF; denominator row to lane 2p+h of dsb
            nc.vector.tensor_copy(yts[p][h * 64:(h + 1) * 64, w * TQW:(w + 1) * TQW],
                                  accs[h][0:HD, :])
            lane = 32 * (2 * p + h)
            nc.vector.tensor_copy(dsb[lane:lane + 1, :], accs[h][HD:HD + 1, :])

        def recip_window(w, half=None):
            # 8 cyc/elem iterative divide on DVE; cost scales with free dim
            sl = slice(0, TQW) if half is None else \
                slice(half * 256, half * 256 + 256)
            nc.vector.reciprocal(rcoll[:, sl], dsb[:, sl])

        def norm_rest(w, half=None):
            sl = slice(0, TQW) if half is None else \
                slice(half * 256, half * 256 + 256)
            n = TQW if half is None else 256
            for p in range(2):
                R = ps.tile([128, TQW], F32, tag="proj", bufs=2, name=f"R{p}{w}{half}")
                nc.tensor.matmul(R[:, 0:n], selw[:, p * 128:(p + 1) * 128],
                                 rcoll[:, sl], start=True, stop=True)
                for h in range(2):
                    yslc = yts[p][h * 64:(h + 1) * 64,
                                  w * TQW + sl.start:w * TQW + sl.stop]
                    nc.vector.tensor_mul(yslc, yslc, R[h * 64:(h + 1) * 64, 0:n])

        def out_tt(tt, dq=None):
            po = ps.tile([128, 1024], F32, tag="st", bufs=2, name=f"po{tt}")
            for nh in range(2):
                for cc in range(2):
                    nc.tensor.matmul(po[:, nh * TQW:(nh + 1) * TQW],
                                     yts[cc][:, tt * 128:(tt + 1) * 128],
                                     wos[:, cc * C + nh * TQW: cc * C + nh * TQW + TQW],
                                     start=(cc == 0), stop=(cc == 1))
            ost = ost_pool.tile([128, 1024], BF16, tag="ost", name=f"o{tt}")
            if isinstance(dq, list):
                # tail: pipeline copy+DMA per 512-col half on two queues
                for nh in range(2):
                    nc.vector.tensor_copy(ost[:, nh * TQW:(nh + 1) * TQW],
                                          po[:, nh * TQW:(nh + 1) * TQW])
                    dq[nh].dma_start(
                        out_ap[tt * 128:(tt + 1) * 128, nh * TQW:(nh + 1) * TQW],
                        ost[:, nh * TQW:(nh + 1) * TQW])
            else:
                nc.vector.tensor_copy(ost[:], po[:])
                deng = nc.sync if dq is None else dq
                deng.dma_start(out_ap[tt * 128:(tt + 1) * 128, :], ost[:])

        # ---------- schedule ----------
        filler = []          # closures of PE work to sprinkle into attention

        def filler_step(n=1):
            for _ in range(n):
                if filler:
                    filler.pop(0)()

        def attn_pair(p, w, per_slot=1):
            ngroups = 2 * (w + 1)
            accs = [ps.tile([128, TQW], F32, tag=f"acc{h}", bufs=1,
                            name=f"acc{p}{w}{h}") for h in range(2)]
            for g in range(ngroups + LAG):
                if g < ngroups:
                    st_slot(p, w, g, 0)
                    st_slot(p, w, g, 1)
                if g >= LAG:
                    gg = g - LAG
                    pv_group(p, w, gg, 0, accs)
                    pv_group(p, w, gg, 1, accs)
                filler_step(per_slot)
            for h in range(2):
                pv_tail(p, w, h, accs)
            if p == 1:
                # start the window's reciprocal as soon as dsb is complete;
                # norm_rest pops as filler a few us later in attn(0,w+1)
                if w < NW - 1:
                    recip_window(w)

        warm()
        # iteration 0: explicit ordering so nothing waits on late DMA
        qk_window(0, 0, 0)
        qk_window(0, 1, 0)
        st_slot(0, 0, 0, 0)
        st_slot(0, 0, 0, 1)
        qk_window(1, 0, 0)
        qk_window(1, 1, 0)
        for tt in range(4):
            v_tile(tt)
        st_slot(0, 0, 1, 0)
        st_slot(0, 0, 1, 1)
        accs0 = [ps.tile([128, TQW], F32, tag=f"acc{h}", bufs=1,
                         name=f"acc00{h}") for h in range(2)]
        for g in range(2):
            pv_group(0, 0, g, 0, accs0)
            pv_group(0, 0, g, 1, accs0)
        for h in range(2):
            pv_tail(0, 0, h, accs0)
        # projections for window 1 run inside pair-1's window-0 attention
        for p in range(2):
            for ty in range(2):
                filler.append(lambda p=p, ty=ty: qk_window(p, ty, 1))
        for tt in range(4, 8):
            filler.append(lambda tt=tt: v_tile(tt))
        attn_pair(1, 0, per_slot=2)  # proj(1) must finish before attn(0,1)
        filler_step(len(filler))
        recip_window(0)

        for w in range(1, NW):
            # filler for this window: projections for slice w+1 (if any),
            # norm of window w-1 (after its recip is done), out of w-1
            items = []
            if w + 1 < NW:
                for p in range(2):
                    for ty in range(2):
                        items.append(lambda p=p, ty=ty, w=w: qk_window(p, ty, w + 1))
                for tt in range(4 * (w + 1), 4 * (w + 1) + 4):
                    items.append(lambda tt=tt: v_tile(tt))
            # norm after 2 proj slots (~2.5us after recip started)
            items.insert(min(2, len(items)), lambda w=w: norm_rest(w - 1))
            pos = 4 if len(items) > 4 else len(items)
            for i, tt in enumerate(range(4 * (w - 1), 4 * (w - 1) + 4)):
                items.insert(pos + 2 * i, lambda tt=tt: out_tt(tt))
            filler.extend(items)
            attn_pair(0, w, per_slot=1)
            attn_pair(1, w, per_slot=1)
            filler_step(len(filler))

        # tail: window 3 normalized + projected in tq halves to pipeline
        # recip / norm / out / DMA; scalar queue is free for DMA here
        recip_window(3, half=0)
        recip_window(3, half=1)
        norm_rest(3, half=0)
        out_tt(12, dq=nc.sync)
        out_tt(13, dq=nc.gpsimd)
        norm_rest(3, half=1)
        out_tt(14, dq=[nc.scalar, nc.gpsimd])
        out_tt(15, dq=[nc.sync, nc.scalar])

    nc.compile()
    return nc


def _selw():
    s = np.zeros((128, 2 * 128), np.float32)
    for p in range(2):
        for h in range(2):
            lane = 32 * (2 * p + h)
            s[lane, p * 128 + h * 64: p * 128 + h * 64 + 64] = 1.0
    return s


def _to_sbuf_chunks(a, nch):
    """[nch*128, F] row-major -> [128, nch*F] SBUF-native layout."""
    n, fdim = a.shape
    assert n == nch * 128
    return np.ascontiguousarray(
        a.reshape(nch, 128, fdim).transpose(1, 0, 2).reshape(128, nch * fdim))


def _prep_core_inputs(b, g, x, Wq, bq, Wk, bk, Wv, bv, Wo, bo):
    bf = ml_dtypes.bfloat16
    f = np.float32
    # x[b].T -> [C,T]; slice-major: [128, (s*NCH+c)*TQW + t']
    xtc = np.ascontiguousarray(x[b].T, dtype=f)          # [C, T]
    xt = (xtc.reshape(NCH, 128, NW, TQW).transpose(1, 2, 0, 3)
          .reshape(128, NW * NCH * TQW)).astype(bf)
    def pack(W, bvec):
        cols = []
        bp = np.empty((2, 128, 1), f)
        for p in range(2):
            h0, h1 = 4 * g + 2 * p, 4 * g + 2 * p + 1
            Wp = np.concatenate([W[:, h0 * HD:(h0 + 1) * HD],
                                 W[:, h1 * HD:(h1 + 1) * HD]], axis=1)
            cols.append(_to_sbuf_chunks(np.ascontiguousarray(Wp, f), NCH))
            bp[p, 0:64, 0] = bvec[h0 * HD:(h0 + 1) * HD]
            bp[p, 64:128, 0] = bvec[h1 * HD:(h1 + 1) * HD]
        return np.concatenate(cols, axis=1).astype(bf), bp
    wq, bqp = pack(Wq, bq)
    wk, bkp = pack(Wk, bk)
    wv = _to_sbuf_chunks(
        np.ascontiguousarray(Wv[:, g * CLOC:(g + 1) * CLOC], f), NCH).astype(bf)
    wo = _to_sbuf_chunks(
        np.ascontiguousarray(Wo[g * CLOC:(g + 1) * CLOC, :], f), 2).astype(bf)
    return {"xt": xt, "wq": wq, "wk": wk, "wv": wv, "wo": wo,
            "bq": bqp, "bk": bkp,
            "ones": np.ones((128, NTT, NHL, 8), bf),
            "selw": _selw(),
            "tri": np.triu(np.ones((128, 128))).astype(bf)}


def _run(inputs, trace=False, tmpdir=None):
    if "nc" not in _cache:
        _cache["nc"] = _build()
    nc = _cache["nc"]
    args = [np.asarray(inputs[k], np.float32) for k in
            ("x", "Wq", "bq", "Wk", "bk", "Wv", "bv", "Wo", "bo")]
    x, Wq, bq, Wk, bk, Wv, bv, Wo, bo = args
    in_maps = [_prep_core_inputs(c // 4, c % 4, x, Wq, bq, Wk, bk, Wv, bv, Wo, bo)
               for c in range(8)]
    res = bass_utils.run_bass_kernel_spmd(nc, in_maps, core_ids=list(range(8)),
                                          trace=trace, tmpdir=tmpdir)
    corr = (bv.astype(np.float64) @ Wo.astype(np.float64) + bo).astype(np.float64)
    out = np.empty((B, T, C), np.float32)
    for b in range(B):
        acc = np.zeros((T, C), np.float64)
        for g in range(4):
            acc += res.results[b * 4 + g]["out"].astype(np.float64)
        out[b] = (acc + corr).astype(np.float32)
    return out, res


def kernel(x, Wq, bq, Wk, bk, Wv, bv, Wo, bo):
    out, _ = _run(dict(x=x, Wq=Wq, bq=bq, Wk=Wk, bk=bk, Wv=Wv, bv=bv,
                       Wo=Wo, bo=bo))
    return out


def run_profiled(x, Wq, bq, Wk, bk, Wv, bv, Wo, bo, tmpdir=None):
    out, res = _run(dict(x=x, Wq=Wq, bq=bq, Wk=Wk, bk=bk, Wv=Wv, bv=bv,
                         Wo=Wo, bo=bo), trace=True, tmpdir=tmpdir)
    return out, res.exec_time_ns, res



# revision 8
# speedup vs baseline: 1.0890x; 1.0890x over previous
"""Causal self-attention kernel for Trainium2, 8-way sharded, fp8 + bf16-clean
window 0.

Problem: B=2, T=2048, C=1024, NH=16, hd=64. fp32 in/out.

Sharding: core = (batch b, head-group g of 4 heads). Each core computes its
4 heads' attention for its batch plus the partial output projection; the
host sums the 4 partials per batch (bv/bo folded via a host-side
correction row, and the weight prescales divided back out).

v5 design (from the v2 bf16 baseline at ~203us):
  - fp8(e4m3) matmuls with DoubleRow perf mode (2 contraction k-tiles per
    pass, 0.5 cyc/row = 4x bf16) for QKV projections, PV, and the output
    projection of windows 1-3. S keeps contraction 64 (head dim): dtype
    doesn't change its 1 cyc/row cost, so q/k are STORED bf16 for free
    accuracy; only the projection operands (x, W) are fp8. Weights are
    prescaled (x16 q/k/v, x64 Wo) to escape fp8 subnormals; the exp scale
    and a host-side divide undo them exactly.
  - fp8 error concentrates in early rows (softmax over few keys -> no
    averaging): rows 0-511 (window 0) use a fully bf16 path: bf16 x
    slice-0 + bf16 Wq/Wk/Wv projections, bf16 P tiles, non-DR bf16 PV
    over a bf16 V copy, bf16 normalized y, bf16 output projection.
    Measured in simulation: all-fp8 = 4.8e-2 max-rel, w0-clean = 4.7e-3.
  - exp(S) on ScalarE (the roofline engine, ~150G elem/s): fp32 PSUM
    score tiles [128,1024] -> P tiles (fp8 for w>=1, bf16 for w0).
    Causal masking after exp: GpSimd memset + tri multiply.
  - Denominators from 8 'ones' columns appended to V (VSTR=72),
    accumulated by the same PV matmuls; batched 4 lanes/window into one
    reciprocal_approx_fast [128,512] (5x cheaper than reciprocal); lane
    -> channel expansion via a small fp32 matmul; unnormalized y kept
    bf16 (fp8 would overflow: y_unnorm ~ 16*y*denom), normalized into
    fp8 (bf16 for w0) by the same DVE pass that applies R.
"""
import contextlib

import ml_dtypes
import numpy as np

import concourse.bass as bass
import concourse.tile as tile
from concourse import bacc, mybir
from concourse import bass_utils

bass_utils.upload_artifacts = lambda tmpdir: "local://skipped"

B, T, C = 2, 2048, 1024
NH, HD = 16, 64
NHL = 4            # heads per core
CLOC = NHL * HD    # 256 local channels
NCH = C // 128     # 8 contraction chunks
NCP = NCH // 2     # 4 chunk pairs (DoubleRow)
TQW = 512          # tq window / T-slice width
NW = T // TQW      # 4 windows
NTT = T // 128     # 16 t-tiles / tk-chunks
VSTR = HD + 8      # 72: v cols per head + 8 ones cols (denom at row 64)
SCL = 16.0         # q/k/v weight prescale (fp8 subnormal escape)
WOSCL = 64.0       # Wo prescale
OUT_DIV = SCL * WOSCL           # host divides partials by this
SEXP = 1.0 / (8.0 * SCL * SCL)  # exp scale: 1/8 softmax * 1/256 q*k descale
F32 = mybir.dt.float32
BF16 = mybir.dt.bfloat16
FP8 = mybir.dt.float8e4
DR = mybir.MatmulPerfMode.DoubleRow
MUL = mybir.AluOpType.mult
EXPF = mybir.ActivationFunctionType.Exp

_cache = {}


def _build(dbg=False):
    nc = bacc.Bacc("TRN2", target_bir_lowering=False, debug=False, num_devices=8)

    xt_ap = nc.dram_tensor("xt", [128, NW * NCH * TQW], FP8, kind="ExternalInput").ap()
    xb_ap = nc.dram_tensor("xb", [128, NCH * TQW], BF16, kind="ExternalInput").ap()
    wq_ap = nc.dram_tensor("wq", [128, 2048], FP8, kind="ExternalInput").ap()
    wk_ap = nc.dram_tensor("wk", [128, 2048], FP8, kind="ExternalInput").ap()
    wv_ap = nc.dram_tensor("wv", [128, 2048], FP8, kind="ExternalInput").ap()
    wo_ap = nc.dram_tensor("wo", [128, 2048], FP8, kind="ExternalInput").ap()
    wqb_ap = nc.dram_tensor("wqb", [128, 2048], BF16, kind="ExternalInput").ap()
    wkb_ap = nc.dram_tensor("wkb", [128, 2048], BF16, kind="ExternalInput").ap()
    wvb_ap = nc.dram_tensor("wvb", [128, 2048], BF16, kind="ExternalInput").ap()
    wob_ap = nc.dram_tensor("wob", [128, 2048], BF16, kind="ExternalInput").ap()
    bq_ap = nc.dram_tensor("bq", [2, 128, 1], F32, kind="ExternalInput").ap()
    bk_ap = nc.dram_tensor("bk", [2, 128, 1], F32, kind="ExternalInput").ap()
    ones_ap = nc.dram_tensor("ones", [128, NTT, NHL, 8], FP8, kind="ExternalInput").ap()
    selw_ap = nc.dram_tensor("selw", [128, 2 * 128], F32, kind="ExternalInput").ap()
    tri_ap = nc.dram_tensor("tri", [128, 128], FP8, kind="ExternalInput").ap()
    trib_ap = nc.dram_tensor("trib", [128, 128], BF16, kind="ExternalInput").ap()
    out_ap = nc.dram_tensor("out", [T, C], BF16, kind="ExternalOutput").ap()

    with tile.TileContext(nc) as tc, contextlib.ExitStack() as ctx:
        sb = ctx.enter_context(tc.tile_pool(name="sb", bufs=1))
        ost_pool = ctx.enter_context(tc.tile_pool(name="ost", bufs=4))
        pt_pool = ctx.enter_context(tc.tile_pool(name="ptp", bufs=8))
        ptb_pool = ctx.enter_context(tc.tile_pool(name="ptbp", bufs=4))
        ps = ctx.enter_context(tc.tile_pool(name="ps", bufs=1, space="PSUM"))

        # ---- persistent SBUF tensors ----
        wqs = sb.tile([128, 2048], FP8, tag="wqs")
        wks = sb.tile([128, 2048], FP8, tag="wks")
        wvs = sb.tile([128, 2048], FP8, tag="wvs")
        wos = sb.tile([128, 2048], FP8, tag="wos")
        wqbs = sb.tile([128, 2048], BF16, tag="wqbs")
        wkbs = sb.tile([128, 2048], BF16, tag="wkbs")
        wvbs = sb.tile([128, 2048], BF16, tag="wvbs")
        wobs = sb.tile([128, 2048], BF16, tag="wobs")
        xts = sb.tile([128, NW * NCH * TQW], FP8, tag="xts")
        xbs = sb.tile([128, NCH * TQW], BF16, tag="xbs")
        qts = [sb.tile([128, T], BF16, tag=f"qt{p}", name=f"qt{p}") for p in range(2)]
        kts = [sb.tile([128, T], BF16, tag=f"kt{p}", name=f"kt{p}") for p in range(2)]
        vna = sb.tile([128, NTT * NHL * VSTR], FP8, tag="vna")
        vnb = sb.tile([128, 4 * NHL * VSTR], BF16, tag="vnb")
        ytu = sb.tile([128, 2 * T], BF16, tag="ytu")   # unnormalized y^T
        yts = sb.tile([128, 2 * T], FP8, tag="yts")    # normalized y^T (w>=1)
        ytsb = sb.tile([128, 2 * TQW], BF16, tag="ytsb")  # normalized y^T (w0)
        bqs = [sb.tile([128, 1], F32, tag=f"bq{p}", name=f"bqs{p}") for p in range(2)]
        bks = [sb.tile([128, 1], F32, tag=f"bk{p}", name=f"bks{p}") for p in range(2)]
        selw = sb.tile([128, 2 * 128], F32, tag="selw")
        dsb = sb.tile([128, TQW], F32, tag="dsb")
        nc.vector.memset(dsb[:], 1.0)
        tri = sb.tile([128, 128], FP8, tag="tri")
        trib = sb.tile([128, 128], BF16, tag="trib")
        rcoll = sb.tile([128, TQW], F32, tag="rcoll")

        vna4 = vna[:].rearrange("p (t h v) -> p t h v", t=NTT, h=NHL)
        vnb4 = vnb[:].rearrange("p (t h v) -> p t h v", t=4, h=NHL)
        nc.vector.memset(vnb4[:, :, :, HD:HD + 8], 1.0)
        xsl = NCH * TQW  # 4096 cols per T-slice

        # ---- input DMAs ----
        # scalar q: w0 bf16 weights first (w0 projections are the critical
        # path), then fp8 weights + consts. sync q: bf16 x slice 0, then fp8
        # slices 1,2. gpsimd q: v weights + ones + fp8 slice 3 + wo.
        for p in range(2):
            nc.scalar.dma_start(bqs[p][:], bq_ap[p])
            nc.scalar.dma_start(bks[p][:], bk_ap[p])
        nc.scalar.dma_start(wqbs[:], wqb_ap[:])
        nc.scalar.dma_start(wkbs[:], wkb_ap[:])
        nc.scalar.dma_start(wqs[:], wq_ap[:])
        nc.scalar.dma_start(wks[:], wk_ap[:])
        nc.scalar.dma_start(selw[:], selw_ap[:])
        nc.scalar.dma_start(tri[:], tri_ap[:])
        nc.scalar.dma_start(trib[:], trib_ap[:])
        nc.sync.dma_start(xbs[:], xb_ap[:])
        nc.sync.dma_start(xts[:, xsl:2 * xsl], xt_ap[:, xsl:2 * xsl])
        nc.sync.dma_start(xts[:, 2 * xsl:3 * xsl], xt_ap[:, 2 * xsl:3 * xsl])
        nc.gpsimd.dma_start(wvbs[:], wvb_ap[:])
        nc.gpsimd.dma_start(wvs[:], wv_ap[:])
        nc.gpsimd.dma_start(vna4[:, :, :, HD:HD + 8], ones_ap[:])
        nc.gpsimd.dma_start(xts[:, 3 * xsl:4 * xsl], xt_ap[:, 3 * xsl:4 * xsl])
        nc.gpsimd.dma_start(wos[:], wo_ap[:])
        nc.sync.dma_start(wobs[:], wob_ap[:])

        # ---------- emission primitives ----------
        def warm():
            wtile = sb.tile([128, 640], BF16, tag="warm")
            wjunk = sb.tile([128, 8], F32, tag="wjunk")
            nc.vector.memset(wtile[:], 0.0)
            wp = ps.tile([128, TQW], F32, tag="proj", bufs=2, name="warm_ps")
            for i in range(14):
                nc.tensor.matmul(wp[:], wtile[:, 0:128], wtile[:, 128:640],
                                 start=True, stop=True)
            nc.vector.tensor_copy(wjunk[:], wp[:, 0:8])
            wact = sb.tile([128, 8], BF16, tag="wact")
            nc.scalar.activation(wact[:], wp[:, 0:8], EXPF, scale=0.125)

        def qk_window(p, ty, w):
            """fp8 DoubleRow projection for windows 1-3."""
            wsb, dst, bias = ((wqs, qts[p], bqs[p]), (wks, kts[p], bks[p]))[ty]
            acc = ps.tile([128, TQW], F32, tag="proj", bufs=2, name=f"qk{p}{ty}{w}")
            for cp in range(NCP):
                lhsT = wsb[:, (p * NCP + cp) * 256:(p * NCP + cp + 1) * 256] \
                    .rearrange("p (j m) -> p j m", j=2)
                rhs = xts[:, (w * NCH + 2 * cp) * TQW:(w * NCH + 2 * cp + 2) * TQW] \
                    .rearrange("p (j t) -> p j t", j=2)
                nc.tensor.matmul(acc[:], lhsT, rhs,
                                 start=(cp == 0), stop=(cp == NCP - 1), perf_mode=DR)
            nc.vector.tensor_scalar_add(dst[:, w * TQW:(w + 1) * TQW], acc[:], bias[:])

        def qk_window0(p, ty):
            """bf16 projection for window 0 (clean path)."""
            wsb, dst, bias = ((wqbs, qts[p], bqs[p]), (wkbs, kts[p], bks[p]))[ty]
            acc = ps.tile([128, TQW], F32, tag="proj", bufs=2, name=f"qb{p}{ty}")
            for c in range(NCH):
                nc.tensor.matmul(acc[:],
                                 wsb[:, (p * NCH + c) * 128:(p * NCH + c + 1) * 128],
                                 xbs[:, c * TQW:(c + 1) * TQW],
                                 start=(c == 0), stop=(c == NCH - 1))
            nc.vector.tensor_scalar_add(dst[:, 0:TQW], acc[:], bias[:])

        def v_tile(tt):
            """fp8 DoubleRow V projection, tiles 4-15."""
            s, q = tt // 4, tt % 4
            acc = ps.tile([128, TQW], F32, tag="proj", bufs=2, name=f"v{tt}")
            for cp in range(NCP):
                lhsT = xts[:, (s * NCH + 2 * cp) * TQW:(s * NCH + 2 * cp + 2) * TQW] \
                    .rearrange("p (j t) -> p j t", j=2)[:, :, q * 128:(q + 1) * 128]
                rhs = wvs[:, cp * 2 * CLOC:(cp + 1) * 2 * CLOC] \
                    .rearrange("p (j n) -> p j n", j=2)
                nc.tensor.matmul(acc[:, 0:CLOC], lhsT, rhs,
                                 start=(cp == 0), stop=(cp == NCP - 1), perf_mode=DR)
            nc.vector.tensor_copy(vna4[:, tt, :, 0:HD],
                                  acc[:, 0:CLOC].rearrange("p (h d) -> p h d", h=NHL))

        def v_tile0(tt):
            """bf16 V projection, tiles 0-3; writes both vnb (bf16, w0 PV) and
            vna (fp8, later windows' PV)."""
            acc = ps.tile([128, TQW], F32, tag="proj", bufs=2, name=f"vb{tt}")
            for c in range(NCH):
                nc.tensor.matmul(acc[:, 0:CLOC],
                                 xbs[:, c * TQW + tt * 128:c * TQW + (tt + 1) * 128],
                                 wvbs[:, c * CLOC:(c + 1) * CLOC],
                                 start=(c == 0), stop=(c == NCH - 1))
            src = acc[:, 0:CLOC].rearrange("p (h d) -> p h d", h=NHL)
            nc.vector.tensor_copy(vnb4[:, tt, :, 0:HD], src)
            nc.vector.tensor_copy(vna4[:, tt, :, 0:HD], src)

        def st_group(h, w, g):
            """S + exp for group g (chunks 2g, 2g+1); masks causal diagonal
            blocks on GpSimd after exp. bf16 P for w0, fp8 otherwise."""
            p, hh = h // 2, h % 2
            kt = kts[p][hh * 64:(hh + 1) * 64, :]
            qt = qts[p][hh * 64:(hh + 1) * 64, :]
            st = ps.tile([128, 1024], F32, tag="st", bufs=2, name=f"st{h}{w}{g}")
            for j in range(2):
                c = 2 * g + j
                nc.tensor.matmul(st[:, j * TQW:(j + 1) * TQW],
                                 kt[:, c * 128:(c + 1) * 128],
                                 qt[:, w * TQW:(w + 1) * TQW],
                                 start=True, stop=True)
            if w == 0:
                pt = ptb_pool.tile([128, 1024], BF16, tag="ptb", name=f"pt{h}{w}{g}")
                trit = trib
            else:
                pt = pt_pool.tile([128, 1024], FP8, tag="pt", name=f"pt{h}{w}{g}")
                trit = tri
            nc.scalar.activation(pt[:], st[:], EXPF, scale=SEXP)
            for j in range(2):
                jp = 2 * g + j - 4 * w
                if jp >= 0:
                    if jp > 0:
                        nc.gpsimd.memset(pt[:, j * TQW:j * TQW + 128 * jp], 0.0)
                    dslc = pt[:, j * TQW + 128 * jp:j * TQW + 128 * jp + 128]
                    nc.gpsimd.tensor_tensor(dslc, dslc, trit[:], op=MUL)
            return pt

        def pv_group(h, w, g, pt, acc):
            if w == 0:
                for j in range(2):
                    c = 2 * g + j
                    nc.tensor.matmul(acc[0:VSTR, :], vnb4[:, c, h, :],
                                     pt[:, j * TQW:(j + 1) * TQW],
                                     start=(c == 0), stop=(c == 3))
            else:
                nc.tensor.matmul(acc[0:VSTR, :], vna4[:, 2 * g:2 * g + 2, h, :],
                                 pt[:].rearrange("p (j t) -> p j t", j=2),
                                 start=(g == 0), stop=(g == 2 * w + 1), perf_mode=DR)

        def pv_tail(h, w, acc):
            p, hh = h // 2, h % 2
            nc.vector.tensor_copy(
                ytu[hh * 64:(hh + 1) * 64, p * T + w * TQW:p * T + (w + 1) * TQW],
                acc[0:HD, :])
            lane = 32 * h
            nc.vector.tensor_copy(dsb[lane:lane + 1, :], acc[HD:HD + 1, :])

        def recip(w, half=None):
            sl = slice(0, TQW) if half is None else \
                slice(half * 256, half * 256 + 256)
            nc.vector.reciprocal_approx_fast(rcoll[:, sl], dsb[:, sl])

        def norm_rest(w, half=None):
            sl = slice(0, TQW) if half is None else \
                slice(half * 256, half * 256 + 256)
            n = TQW if half is None else 256
            for p in range(2):
                R = ps.tile([128, TQW], F32, tag="proj", bufs=2,
                            name=f"R{p}{w}{half}")
                nc.tensor.matmul(R[:, 0:n],
                                 selw[:, p * 128:(p + 1) * 128],
                                 rcoll[:, sl], start=True, stop=True)
                cb = p * T + w * TQW + sl.start
                if w == 0:
                    nc.vector.tensor_tensor(
                        ytsb[:, p * TQW + sl.start:p * TQW + sl.start + n],
                        ytu[:, cb:cb + n], R[:, 0:n], op=MUL)
                else:
                    nc.vector.tensor_tensor(yts[:, cb:cb + n], ytu[:, cb:cb + n],
                                            R[:, 0:n], op=MUL)

        yts2 = yts[:].rearrange("p (j t) -> p j t", j=2)
        wos2 = wos[:].rearrange("p (j n) -> p j n", j=2)

        def out_tt(tt, half, dq=None):
            po = ps.tile([128, TQW], F32, tag="proj", bufs=2,
                         name=f"po{tt}{half}")
            if tt < 4:
                for p in range(2):
                    nc.tensor.matmul(
                        po[:],
                        ytsb[:, p * TQW + tt * 128:p * TQW + (tt + 1) * 128],
                        wobs[:, p * 1024 + half * TQW:p * 1024 + (half + 1) * TQW],
                        start=(p == 0), stop=(p == 1))
            else:
                nc.tensor.matmul(po[:], yts2[:, :, tt * 128:(tt + 1) * 128],
                                 wos2[:, :, half * TQW:(half + 1) * TQW],
                                 start=True, stop=True, perf_mode=DR)
            ost = ost_pool.tile([128, TQW], BF16, tag="ost", name=f"o{tt}{half}")
            nc.vector.tensor_copy(ost[:], po[:])
            (dq or nc.sync).dma_start(
                out_ap[tt * 128:(tt + 1) * 128, half * TQW:(half + 1) * TQW],
                ost[:])

        # ---------- schedule ----------
        filler = []

        def filler_step(n=1):
            for _ in range(n):
                if filler:
                    filler.pop(0)()

        LAG = 2

        def attn_head(h, w, per_slot=1):
            ngroups = 2 * w + 2
            acc = ps.tile([128, TQW], F32, tag="acc", bufs=2, name=f"acc{h}{w}")
            pts = {}
            for g in range(ngroups + LAG):
                if g < ngroups:
                    pts[g] = st_group(h, w, g)
                if g >= LAG:
                    gg = g - LAG
                    pv_group(h, w, gg, pts.pop(gg), acc)
                filler_step(per_slot)
            pv_tail(h, w, acc)

        warm()
        qk_window0(0, 0)
        qk_window0(0, 1)
        for tt in range(4):
            v_tile0(tt)
        qk_window0(1, 0)
        qk_window0(1, 1)
        # window 0; window-1 projections + v tiles 4..7 interleave as filler
        for p in range(2):
            for ty in range(2):
                filler.append(lambda p=p, ty=ty: qk_window(p, ty, 1))
        for tt in range(4, 8):
            filler.append(lambda tt=tt: v_tile(tt))
        for h in range(4):
            attn_head(h, 0, per_slot=1)
        recip(0)
        filler_step(len(filler))

        for w in range(1, NW):
            items = []
            if w + 1 < NW:
                for p in range(2):
                    for ty in range(2):
                        items.append(lambda p=p, ty=ty, w=w: qk_window(p, ty, w + 1))
                for tt in range(4 * (w + 1), 4 * (w + 1) + 4):
                    items.append(lambda tt=tt: v_tile(tt))
            items.insert(min(2, len(items)), lambda w=w: norm_rest(w - 1))
            pos = 4 if len(items) > 4 else len(items)
            outs = []
            for tt in range(4 * (w - 1), 4 * (w - 1) + 4):
                for half in range(2):
                    outs.append(lambda tt=tt, half=half: out_tt(tt, half))
            for i, fn in enumerate(outs):
                items.insert(pos + i, fn)
            filler.extend(items)
            for h in range(4):
                attn_head(h, w, per_slot=1)
            if w < NW - 1:
                recip(w)
            filler_step(len(filler))

        # tail: window 3 normalized + projected in halves to pipeline
        recip(3, half=0)
        recip(3, half=1)
        norm_rest(3, half=0)
        out_tt(12, 0, dq=nc.sync)
        out_tt(12, 1, dq=nc.gpsimd)
        out_tt(13, 0, dq=nc.scalar)
        norm_rest(3, half=1)
        out_tt(13, 1, dq=nc.sync)
        out_tt(14, 0, dq=nc.scalar)
        out_tt(14, 1, dq=nc.gpsimd)
        out_tt(15, 0, dq=nc.sync)
        out_tt(15, 1, dq=nc.scalar)

    nc.compile()
    return nc


def _selw():
    s = np.zeros((128, 2 * 128), np.float32)
    for p in range(2):
        for hh in range(2):
            lane = 32 * (2 * p + hh)
            s[lane, p * 128 + hh * 64: p * 128 + hh * 64 + 64] = 1.0
    return s


def _to_sbuf_chunks(a, nch):
    """[nch*128, F] row-major -> [128, nch*F] SBUF-native layout."""
    n, fdim = a.shape
    assert n == nch * 128
    return np.ascontiguousarray(
        a.reshape(nch, 128, fdim).transpose(1, 0, 2).reshape(128, nch * fdim))


def _prep_core_inputs(b, g, x, Wq, bq, Wk, bk, Wv, bv, Wo, bo):
    f8 = ml_dtypes.float8_e4m3
    bf = ml_dtypes.bfloat16
    f = np.float32
    # x[b].T -> [C,T]; slice-major: [128, (s*NCH+c)*TQW + t']
    xtc = np.ascontiguousarray(x[b].T, dtype=f)          # [C, T]
    xsl = (xtc.reshape(NCH, 128, NW, TQW).transpose(1, 2, 0, 3)
           .reshape(128, NW * NCH * TQW))
    xt = xsl.astype(f8)
    xb = np.ascontiguousarray(xsl[:, 0:NCH * TQW]).astype(bf)

    def pack_qk_pair(W, bvec):
        """Per-pair weight cols [C, 128] (m = hh*64+d) and scaled bias."""
        Wp = []
        bp = np.empty((2, 128, 1), f)
        for p in range(2):
            h0, h1 = 4 * g + 2 * p, 4 * g + 2 * p + 1
            Wp.append(np.concatenate([W[:, h0 * HD:(h0 + 1) * HD],
                                      W[:, h1 * HD:(h1 + 1) * HD]], axis=1) * SCL)
            bp[p, 0:64, 0] = bvec[h0 * HD:(h0 + 1) * HD] * SCL
            bp[p, 64:128, 0] = bvec[h1 * HD:(h1 + 1) * HD] * SCL
        return Wp, bp

    def pack_dr(Wp):
        # [128, 2048]: col = ((p*NCP+cp)*2 + j)*128 + m
        out = np.empty((128, 2048), f)
        for p in range(2):
            for cp in range(NCP):
                for j in range(2):
                    c = 2 * cp + j
                    blk = (p * NCP + cp) * 2 + j
                    out[:, blk * 128:(blk + 1) * 128] = \
                        Wp[p][c * 128:(c + 1) * 128, :]
        return out.astype(f8)

    def pack_plain(Wp):
        # [128, 2048]: col = p*1024 + c*128 + m
        return np.concatenate(
            [_to_sbuf_chunks(np.ascontiguousarray(Wp[p], f), NCH)
             for p in range(2)], axis=1).astype(bf)

    Wqp, bqp = pack_qk_pair(Wq, bq)
    Wkp, bkp = pack_qk_pair(Wk, bk)
    Wvl = np.ascontiguousarray(Wv[:, g * CLOC:(g + 1) * CLOC], f) * SCL
    wv_chunks = _to_sbuf_chunks(Wvl, NCH)
    Wol = np.ascontiguousarray(Wo[g * CLOC:(g + 1) * CLOC, :], f) * WOSCL
    wo_chunks = _to_sbuf_chunks(Wol, 2)

    return {"xt": xt, "xb": xb,
            "wq": pack_dr(Wqp), "wk": pack_dr(Wkp),
            "wqb": pack_plain(Wqp), "wkb": pack_plain(Wkp),
            "wv": wv_chunks.astype(f8), "wvb": wv_chunks.astype(bf),
            "wo": wo_chunks.astype(f8), "wob": wo_chunks.astype(bf),
            "bq": bqp, "bk": bkp,
            "ones": np.ones((128, NTT, NHL, 8), f8),
            "selw": _selw(),
            "tri": np.triu(np.ones((128, 128))).astype(f8),
            "trib": np.triu(np.ones((128, 128))).astype(bf)}


def _run(inputs, trace=False, tmpdir=None):
    if "nc" not in _cache:
        _cache["nc"] = _build()
    nc = _cache["nc"]
    args = [np.asarray(inputs[k], np.float32) for k in
            ("x", "Wq", "bq", "Wk", "bk", "Wv", "bv", "Wo", "bo")]
    x, Wq, bq, Wk, bk, Wv, bv, Wo, bo = args
    in_maps = [_prep_core_inputs(c // 4, c % 4, x, Wq, bq, Wk, bk, Wv, bv, Wo, bo)
               for c in range(8)]
    res = bass_utils.run_bass_kernel_spmd(nc, in_maps, core_ids=list(range(8)),
                                          trace=trace, tmpdir=tmpdir)
    corr = (bv.astype(np.float64) @ Wo.astype(np.float64) + bo).astype(np.float64)
    out = np.empty((B, T, C), np.float32)
    for b in range(B):
        acc = np.zeros((T, C), np.float64)
        for g in range(4):
            acc += res.results[b * 4 + g]["out"].astype(np.float64)
        out[b] = (acc / OUT_DIV + corr).astype(np.float32)
    return out, res


def kernel(x, Wq, bq, Wk, bk, Wv, bv, Wo, bo):
    out, _ = _run(dict(x=x, Wq=Wq, bq=bq, Wk=Wk, bk=bk, Wv=Wv, bv=bv,
                       Wo=Wo, bo=bo))
    return out


def run_profiled(x, Wq, bq, Wk, bk, Wv, bv, Wo, bo, tmpdir=None):
    out, res = _run(dict(x=x, Wq=Wq, bq=bq, Wk=Wk, bk=bk, Wv=Wv, bv=bv,
                         Wo=Wo, bo=bo), trace=True, tmpdir=tmpdir)
    return out, res.exec_time_ns, res


# revision 10
# speedup vs baseline: 1.1539x; 1.0596x over previous
"""Causal self-attention kernel for Trainium2, 8-way sharded, fp8 + bf16-clean
window 0.

Problem: B=2, T=2048, C=1024, NH=16, hd=64. fp32 in/out.

Sharding: core = (batch b, head-group g of 4 heads). Each core computes its
4 heads' attention for its batch plus the partial output projection; the
host sums the 4 partials per batch (bv/bo folded via a host-side
correction row, and the weight prescales divided back out).

v5 design (from the v2 bf16 baseline at ~203us):
  - fp8(e4m3) matmuls with DoubleRow perf mode (2 contraction k-tiles per
    pass, 0.5 cyc/row = 4x bf16) for QKV projections, PV, and the output
    projection of windows 1-3. S keeps contraction 64 (head dim): dtype
    doesn't change its 1 cyc/row cost, so q/k are STORED bf16 for free
    accuracy; only the projection operands (x, W) are fp8. Weights are
    prescaled (x16 q/k/v, x64 Wo) to escape fp8 subnormals; the exp scale
    and a host-side divide undo them exactly.
  - fp8 error concentrates in early rows (softmax over few keys -> no
    averaging): rows 0-511 (window 0) use a fully bf16 path: bf16 x
    slice-0 + bf16 Wq/Wk/Wv projections, bf16 P tiles, non-DR bf16 PV
    over a bf16 V copy, bf16 normalized y, bf16 output projection.
    Measured in simulation: all-fp8 = 4.8e-2 max-rel, w0-clean = 4.7e-3.
  - exp(S) on ScalarE (the roofline engine, ~150G elem/s): fp32 PSUM
    score tiles [128,1024] -> P tiles (fp8 for w>=1, bf16 for w0).
    Causal masking after exp: GpSimd memset + tri multiply.
  - Denominators from 8 'ones' columns appended to V (VSTR=72),
    accumulated by the same PV matmuls; batched 4 lanes/window into one
    reciprocal_approx_fast [128,512] (5x cheaper than reciprocal); lane
    -> channel expansion via a small fp32 matmul; unnormalized y kept
    bf16 (fp8 would overflow: y_unnorm ~ 16*y*denom), normalized into
    fp8 (bf16 for w0) by the same DVE pass that applies R.
"""
import contextlib

import ml_dtypes
import numpy as np

import concourse.bass as bass
import concourse.tile as tile
from concourse import bacc, mybir
from concourse import bass_utils

bass_utils.upload_artifacts = lambda tmpdir: "local://skipped"

B, T, C = 2, 2048, 1024
NH, HD = 16, 64
NHL = 4            # heads per core
CLOC = NHL * HD    # 256 local channels
NCH = C // 128     # 8 contraction chunks
NCP = NCH // 2     # 4 chunk pairs (DoubleRow)
TQW = 512          # tq window / T-slice width
NW = T // TQW      # 4 windows
NTT = T // 128     # 16 t-tiles / tk-chunks
VSTR = HD + 8      # 72: v cols per head + 8 ones cols (denom at row 64)
SCL = 16.0         # q/k/v weight prescale (fp8 subnormal escape)
WOSCL = 64.0       # Wo prescale
OUT_DIV = SCL * WOSCL           # host divides partials by this
SEXP = 1.0 / (8.0 * SCL * SCL)  # exp scale: 1/8 softmax * 1/256 q*k descale
F32 = mybir.dt.float32
BF16 = mybir.dt.bfloat16
FP8 = mybir.dt.float8e4
DR = mybir.MatmulPerfMode.DoubleRow
MUL = mybir.AluOpType.mult
EXPF = mybir.ActivationFunctionType.Exp

_cache = {}


def _build(dbg=False):
    nc = bacc.Bacc("TRN2", target_bir_lowering=False, debug=False, num_devices=8)

    xt_ap = nc.dram_tensor("xt", [128, NW * NCH * TQW], FP8, kind="ExternalInput").ap()
    xb_ap = nc.dram_tensor("xb", [128, NCH * TQW], BF16, kind="ExternalInput").ap()
    wq_ap = nc.dram_tensor("wq", [128, 2048], FP8, kind="ExternalInput").ap()
    wk_ap = nc.dram_tensor("wk", [128, 2048], FP8, kind="ExternalInput").ap()
    wv_ap = nc.dram_tensor("wv", [128, 2048], FP8, kind="ExternalInput").ap()
    wo_ap = nc.dram_tensor("wo", [128, 2048], FP8, kind="ExternalInput").ap()
    wqb_ap = nc.dram_tensor("wqb", [128, 2048], BF16, kind="ExternalInput").ap()
    wkb_ap = nc.dram_tensor("wkb", [128, 2048], BF16, kind="ExternalInput").ap()
    wvb_ap = nc.dram_tensor("wvb", [128, 2048], BF16, kind="ExternalInput").ap()
    wob_ap = nc.dram_tensor("wob", [128, 2048], BF16, kind="ExternalInput").ap()
    bq_ap = nc.dram_tensor("bq", [2, 128, 1], F32, kind="ExternalInput").ap()
    bk_ap = nc.dram_tensor("bk", [2, 128, 1], F32, kind="ExternalInput").ap()
    ones_ap = nc.dram_tensor("ones", [128, NTT, NHL, 8], FP8, kind="ExternalInput").ap()
    selw_ap = nc.dram_tensor("selw", [128, 2 * 128], BF16, kind="ExternalInput").ap()
    tri_ap = nc.dram_tensor("tri", [128, 128], FP8, kind="ExternalInput").ap()
    trib_ap = nc.dram_tensor("trib", [128, 128], BF16, kind="ExternalInput").ap()
    out_ap = nc.dram_tensor("out", [T, C], BF16, kind="ExternalOutput").ap()

    with tile.TileContext(nc) as tc, contextlib.ExitStack() as ctx:
        sb = ctx.enter_context(tc.tile_pool(name="sb", bufs=1))
        ost_pool = ctx.enter_context(tc.tile_pool(name="ost", bufs=4))
        pt_pool = ctx.enter_context(tc.tile_pool(name="ptp", bufs=8))
        ptb_pool = ctx.enter_context(tc.tile_pool(name="ptbp", bufs=4))
        ps = ctx.enter_context(tc.tile_pool(name="ps", bufs=1, space="PSUM"))

        # ---- persistent SBUF tensors ----
        wqs = sb.tile([128, 2048], FP8, tag="wqs")
        wks = sb.tile([128, 2048], FP8, tag="wks")
        wvs = sb.tile([128, 2048], FP8, tag="wvs")
        wos = sb.tile([128, 2048], FP8, tag="wos")
        wqbs = sb.tile([128, 2048], BF16, tag="wqbs")
        wkbs = sb.tile([128, 2048], BF16, tag="wkbs")
        wvbs = sb.tile([128, 2048], BF16, tag="wvbs")
        wobs = sb.tile([128, 2048], BF16, tag="wobs")
        xts = sb.tile([128, NW * NCH * TQW], FP8, tag="xts")
        xbs = sb.tile([128, NCH * TQW], BF16, tag="xbs")
        qts = [sb.tile([128, T], BF16, tag=f"qt{p}", name=f"qt{p}") for p in range(2)]
        kts = [sb.tile([128, T], BF16, tag=f"kt{p}", name=f"kt{p}") for p in range(2)]
        vna = sb.tile([128, NTT * NHL * VSTR], FP8, tag="vna")
        vnb = sb.tile([128, 4 * NHL * VSTR], BF16, tag="vnb")
        ytu = sb.tile([128, 2 * T], BF16, tag="ytu")   # unnormalized y^T
        yts = sb.tile([128, 2 * T], FP8, tag="yts")    # normalized y^T (w>=1)
        ytsb = sb.tile([128, 2 * TQW], BF16, tag="ytsb")  # normalized y^T (w0)
        bqs = [sb.tile([128, 1], F32, tag=f"bq{p}", name=f"bqs{p}") for p in range(2)]
        bks = [sb.tile([128, 1], F32, tag=f"bk{p}", name=f"bks{p}") for p in range(2)]
        selw = sb.tile([128, 2 * 128], BF16, tag="selw")
        dsb = sb.tile([128, TQW], F32, tag="dsb")
        nc.vector.memset(dsb[:], 1.0)
        tri = sb.tile([128, 128], FP8, tag="tri")
        trib = sb.tile([128, 128], BF16, tag="trib")
        rcoll = sb.tile([128, TQW], F32, tag="rcoll")
        rcb = sb.tile([128, TQW], BF16, tag="rcb")

        vna4 = vna[:].rearrange("p (t h v) -> p t h v", t=NTT, h=NHL)
        vnb4 = vnb[:].rearrange("p (t h v) -> p t h v", t=4, h=NHL)
        nc.vector.memset(vnb4[:, :, :, HD:HD + 8], 1.0)
        xsl = NCH * TQW  # 4096 cols per T-slice

        # ---- input DMAs ----
        # scalar q: w0 bf16 weights first (w0 projections are the critical
        # path), then fp8 weights + consts. sync q: bf16 x slice 0, then fp8
        # slices 1,2. gpsimd q: v weights + ones + fp8 slice 3 + wo.
        for p in range(2):
            nc.scalar.dma_start(bqs[p][:], bq_ap[p])
            nc.scalar.dma_start(bks[p][:], bk_ap[p])
        nc.scalar.dma_start(selw[:], selw_ap[:])
        nc.scalar.dma_start(tri[:], tri_ap[:])
        nc.scalar.dma_start(trib[:], trib_ap[:])
        nc.scalar.dma_start(wqbs[:, 0:1024], wqb_ap[:, 0:1024])
        nc.scalar.dma_start(wkbs[:, 0:1024], wkb_ap[:, 0:1024])
        nc.scalar.dma_start(wqbs[:, 1024:2048], wqb_ap[:, 1024:2048])
        nc.scalar.dma_start(wkbs[:, 1024:2048], wkb_ap[:, 1024:2048])
        nc.scalar.dma_start(wqs[:], wq_ap[:])
        nc.scalar.dma_start(wks[:], wk_ap[:])
        nc.sync.dma_start(xbs[:, 0:4 * TQW], xb_ap[:, 0:4 * TQW])
        nc.sync.dma_start(xbs[:, 4 * TQW:8 * TQW], xb_ap[:, 4 * TQW:8 * TQW])
        nc.sync.dma_start(xts[:, xsl:2 * xsl], xt_ap[:, xsl:2 * xsl])
        nc.sync.dma_start(xts[:, 2 * xsl:3 * xsl], xt_ap[:, 2 * xsl:3 * xsl])
        nc.sync.dma_start(wobs[:], wob_ap[:])
        nc.gpsimd.dma_start(wvbs[:], wvb_ap[:])
        nc.gpsimd.dma_start(wvs[:], wv_ap[:])
        nc.gpsimd.dma_start(vna4[:, :, :, HD:HD + 8], ones_ap[:])
        nc.gpsimd.dma_start(xts[:, 3 * xsl:4 * xsl], xt_ap[:, 3 * xsl:4 * xsl])
        nc.gpsimd.dma_start(wos[:], wo_ap[:])

        # ---------- emission primitives ----------
        def warm():
            wtile = sb.tile([128, 640], BF16, tag="warm")
            wjunk = sb.tile([128, 8], F32, tag="wjunk")
            nc.vector.memset(wtile[:], 0.0)
            wp = ps.tile([128, TQW], F32, tag="proj", bufs=2, name="warm_ps")
            for i in range(14):
                nc.tensor.matmul(wp[:], wtile[:, 0:128], wtile[:, 128:640],
                                 start=True, stop=True)
            nc.vector.tensor_copy(wjunk[:], wp[:, 0:8])
            wact = sb.tile([128, 8], BF16, tag="wact")
            nc.scalar.activation(wact[:], wp[:, 0:8], EXPF, scale=0.125)

        def qk_window(p, ty, w):
            """fp8 DoubleRow projection for windows 1-3."""
            wsb, dst, bias = ((wqs, qts[p], bqs[p]), (wks, kts[p], bks[p]))[ty]
            acc = ps.tile([128, TQW], F32, tag="proj", bufs=2, name=f"qk{p}{ty}{w}")
            for cp in range(NCP):
                lhsT = wsb[:, (p * NCP + cp) * 256:(p * NCP + cp + 1) * 256] \
                    .rearrange("p (j m) -> p j m", j=2)
                rhs = xts[:, (w * NCH + 2 * cp) * TQW:(w * NCH + 2 * cp + 2) * TQW] \
                    .rearrange("p (j t) -> p j t", j=2)
                nc.tensor.matmul(acc[:], lhsT, rhs,
                                 start=(cp == 0), stop=(cp == NCP - 1), perf_mode=DR)
            nc.vector.tensor_scalar_add(dst[:, w * TQW:(w + 1) * TQW], acc[:], bias[:])

        def qk_window0(p, ty):
            """bf16 projection for window 0 (clean path)."""
            wsb, dst, bias = ((wqbs, qts[p], bqs[p]), (wkbs, kts[p], bks[p]))[ty]
            acc = ps.tile([128, TQW], F32, tag="proj", bufs=2, name=f"qb{p}{ty}")
            for c in range(NCH):
                nc.tensor.matmul(acc[:],
                                 wsb[:, (p * NCH + c) * 128:(p * NCH + c + 1) * 128],
                                 xbs[:, c * TQW:(c + 1) * TQW],
                                 start=(c == 0), stop=(c == NCH - 1))
            nc.vector.tensor_scalar_add(dst[:, 0:TQW], acc[:], bias[:])

        def v_tile(tt):
            """fp8 DoubleRow V projection, tiles 4-15."""
            s, q = tt // 4, tt % 4
            acc = ps.tile([128, TQW], F32, tag="proj", bufs=2, name=f"v{tt}")
            for cp in range(NCP):
                lhsT = xts[:, (s * NCH + 2 * cp) * TQW:(s * NCH + 2 * cp + 2) * TQW] \
                    .rearrange("p (j t) -> p j t", j=2)[:, :, q * 128:(q + 1) * 128]
                rhs = wvs[:, cp * 2 * CLOC:(cp + 1) * 2 * CLOC] \
                    .rearrange("p (j n) -> p j n", j=2)
                nc.tensor.matmul(acc[:, 0:CLOC], lhsT, rhs,
                                 start=(cp == 0), stop=(cp == NCP - 1), perf_mode=DR)
            nc.vector.tensor_copy(vna4[:, tt, :, 0:HD],
                                  acc[:, 0:CLOC].rearrange("p (h d) -> p h d", h=NHL))

        def v_tile0(tt):
            """bf16 V projection, tiles 0-3; writes both vnb (bf16, w0 PV) and
            vna (fp8, later windows' PV)."""
            acc = ps.tile([128, TQW], F32, tag="proj", bufs=2, name=f"vb{tt}")
            for c in range(NCH):
                nc.tensor.matmul(acc[:, 0:CLOC],
                                 xbs[:, c * TQW + tt * 128:c * TQW + (tt + 1) * 128],
                                 wvbs[:, c * CLOC:(c + 1) * CLOC],
                                 start=(c == 0), stop=(c == NCH - 1))
            src = acc[:, 0:CLOC].rearrange("p (h d) -> p h d", h=NHL)
            nc.vector.tensor_copy(vnb4[:, tt, :, 0:HD], src)
            nc.vector.tensor_copy(vna4[:, tt, :, 0:HD], src)

        def st_group(h, w, g):
            """S + exp for group g (chunks 2g, 2g+1); masks causal diagonal
            blocks on GpSimd after exp. bf16 P for w0, fp8 otherwise."""
            p, hh = h // 2, h % 2
            kt = kts[p][hh * 64:(hh + 1) * 64, :]
            qt = qts[p][hh * 64:(hh + 1) * 64, :]
            st = ps.tile([128, 1024], F32, tag="st", bufs=2, name=f"st{h}{w}{g}")
            for j in range(2):
                c = 2 * g + j
                nc.tensor.matmul(st[:, j * TQW:(j + 1) * TQW],
                                 kt[:, c * 128:(c + 1) * 128],
                                 qt[:, w * TQW:(w + 1) * TQW],
                                 start=True, stop=True)
            if w == 0:
                pt = ptb_pool.tile([128, 1024], BF16, tag="ptb", name=f"pt{h}{w}{g}")
                trit = trib
            else:
                pt = pt_pool.tile([128, 1024], FP8, tag="pt", name=f"pt{h}{w}{g}")
                trit = tri
            nc.scalar.activation(pt[:], st[:], EXPF, scale=SEXP)
            for j in range(2):
                jp = 2 * g + j - 4 * w
                if jp >= 0:
                    if jp > 0:
                        nc.gpsimd.memset(pt[:, j * TQW:j * TQW + 128 * jp], 0.0)
                    dslc = pt[:, j * TQW + 128 * jp:j * TQW + 128 * jp + 128]
                    nc.gpsimd.tensor_tensor(dslc, dslc, trit[:], op=MUL)
            return pt

        def pv_group(h, w, g, pt, acc):
            if w == 0:
                for j in range(2):
                    c = 2 * g + j
                    nc.tensor.matmul(acc[0:VSTR, :], vnb4[:, c, h, :],
                                     pt[:, j * TQW:(j + 1) * TQW],
                                     start=(c == 0), stop=(c == 3))
            else:
                nc.tensor.matmul(acc[0:VSTR, :], vna4[:, 2 * g:2 * g + 2, h, :],
                                 pt[:].rearrange("p (j t) -> p j t", j=2),
                                 start=(g == 0), stop=(g == 2 * w + 1), perf_mode=DR)

        def pv_tail(h, w, acc):
            p, hh = h // 2, h % 2
            nc.vector.tensor_copy(
                ytu[hh * 64:(hh + 1) * 64, p * T + w * TQW:p * T + (w + 1) * TQW],
                acc[0:HD, :])
            lane = 32 * h
            nc.vector.tensor_copy(dsb[lane:lane + 1, :], acc[HD:HD + 1, :])

        def recip(w, half=None):
            sl = slice(0, TQW) if half is None else \
                slice(half * 256, half * 256 + 256)
            nc.vector.reciprocal_approx_fast(rcoll[:, sl], dsb[:, sl])
            nc.vector.tensor_copy(rcb[:, sl], rcoll[:, sl])

        def norm_rest(w, half=None):
            sl = slice(0, TQW) if half is None else \
                slice(half * 256, half * 256 + 256)
            n = TQW if half is None else 256
            for p in range(2):
                R = ps.tile([128, TQW], F32, tag="proj", bufs=2,
                            name=f"R{p}{w}{half}")
                nc.tensor.matmul(R[:, 0:n],
                                 selw[:, p * 128:(p + 1) * 128],
                                 rcb[:, sl], start=True, stop=True)
                cb = p * T + w * TQW + sl.start
                if w == 0:
                    nc.vector.tensor_tensor(
                        ytsb[:, p * TQW + sl.start:p * TQW + sl.start + n],
                        ytu[:, cb:cb + n], R[:, 0:n], op=MUL)
                else:
                    nc.vector.tensor_tensor(yts[:, cb:cb + n], ytu[:, cb:cb + n],
                                            R[:, 0:n], op=MUL)

        yts2 = yts[:].rearrange("p (j t) -> p j t", j=2)
        wos2 = wos[:].rearrange("p (j n) -> p j n", j=2)

        def out_tt(tt, half, dq=None):
            po = ps.tile([128, TQW], F32, tag="proj", bufs=2,
                         name=f"po{tt}{half}")
            if tt < 4:
                for p in range(2):
                    nc.tensor.matmul(
                        po[:],
                        ytsb[:, p * TQW + tt * 128:p * TQW + (tt + 1) * 128],
                        wobs[:, p * 1024 + half * TQW:p * 1024 + (half + 1) * TQW],
                        start=(p == 0), stop=(p == 1))
            else:
                nc.tensor.matmul(po[:], yts2[:, :, tt * 128:(tt + 1) * 128],
                                 wos2[:, :, half * TQW:(half + 1) * TQW],
                                 start=True, stop=True, perf_mode=DR)
            ost = ost_pool.tile([128, TQW], BF16, tag="ost", name=f"o{tt}{half}")
            nc.vector.tensor_copy(ost[:], po[:])
            (dq or nc.sync).dma_start(
                out_ap[tt * 128:(tt + 1) * 128, half * TQW:(half + 1) * TQW],
                ost[:])

        # ---------- schedule ----------
        filler = []

        def filler_step(n=1):
            for _ in range(n):
                if filler:
                    filler.pop(0)()

        LAG = 2

        def attn_head(h, w, per_slot=1):
            ngroups = 2 * w + 2
            acc = ps.tile([128, TQW], F32, tag="acc", bufs=2, name=f"acc{h}{w}")
            pts = {}
            for g in range(ngroups + LAG):
                if g < ngroups:
                    pts[g] = st_group(h, w, g)
                if g >= LAG:
                    gg = g - LAG
                    pv_group(h, w, gg, pts.pop(gg), acc)
                filler_step(per_slot)
            pv_tail(h, w, acc)

        warm()
        qk_window0(0, 0)
        qk_window0(0, 1)
        for tt in range(4):
            v_tile0(tt)
        qk_window0(1, 0)
        qk_window0(1, 1)
        # window 0; window-1 projections + v tiles 4..7 interleave as filler
        for p in range(2):
            for ty in range(2):
                filler.append(lambda p=p, ty=ty: qk_window(p, ty, 1))
        for tt in range(4, 8):
            filler.append(lambda tt=tt: v_tile(tt))
        for h in range(4):
            attn_head(h, 0, per_slot=1)
        recip(0)
        filler_step(len(filler))

        for w in range(1, NW):
            items = []
            if w + 1 < NW:
                for p in range(2):
                    for ty in range(2):
                        items.append(lambda p=p, ty=ty, w=w: qk_window(p, ty, w + 1))
                for tt in range(4 * (w + 1), 4 * (w + 1) + 4):
                    items.append(lambda tt=tt: v_tile(tt))
            items.insert(min(2, len(items)), lambda w=w: norm_rest(w - 1))
            pos = 4 if len(items) > 4 else len(items)
            outs = []
            for tt in range(4 * (w - 1), 4 * (w - 1) + 4):
                for half in range(2):
                    outs.append(lambda tt=tt, half=half: out_tt(tt, half))
            for i, fn in enumerate(outs):
                items.insert(pos + i, fn)
            filler.extend(items)
            for h in range(4):
                attn_head(h, w, per_slot=1)
            if w < NW - 1:
                recip(w)
            filler_step(len(filler))

        # tail: window 3 normalized + projected in halves to pipeline
        recip(3, half=0)
        recip(3, half=1)
        norm_rest(3, half=0)
        out_tt(12, 0, dq=nc.sync)
        out_tt(12, 1, dq=nc.gpsimd)
        out_tt(13, 0, dq=nc.scalar)
        norm_rest(3, half=1)
        out_tt(13, 1, dq=nc.sync)
        out_tt(14, 0, dq=nc.scalar)
        out_tt(14, 1, dq=nc.gpsimd)
        out_tt(15, 0, dq=nc.sync)
        out_tt(15, 1, dq=nc.scalar)

    nc.compile()
    return nc


def _selw():
    s = np.zeros((128, 2 * 128), np.float32)
    for p in range(2):
        for hh in range(2):
            lane = 32 * (2 * p + hh)
            s[lane, p * 128 + hh * 64: p * 128 + hh * 64 + 64] = 1.0
    return s


def _to_sbuf_chunks(a, nch):
    """[nch*128, F] row-major -> [128, nch*F] SBUF-native layout."""
    n, fdim = a.shape
    assert n == nch * 128
    return np.ascontiguousarray(
        a.reshape(nch, 128, fdim).transpose(1, 0, 2).reshape(128, nch * fdim))


def _prep_core_inputs(b, g, x, Wq, bq, Wk, bk, Wv, bv, Wo, bo):
    f8 = ml_dtypes.float8_e4m3
    bf = ml_dtypes.bfloat16
    f = np.float32
    # x[b].T -> [C,T]; slice-major: [128, (s*NCH+c)*TQW + t']
    xtc = np.ascontiguousarray(x[b].T, dtype=f)          # [C, T]
    xsl = (xtc.reshape(NCH, 128, NW, TQW).transpose(1, 2, 0, 3)
           .reshape(128, NW * NCH * TQW))
    xt = xsl.astype(f8)
    xb = np.ascontiguousarray(xsl[:, 0:NCH * TQW]).astype(bf)

    def pack_qk_pair(W, bvec):
        """Per-pair weight cols [C, 128] (m = hh*64+d) and scaled bias."""
        Wp = []
        bp = np.empty((2, 128, 1), f)
        for p in range(2):
            h0, h1 = 4 * g + 2 * p, 4 * g + 2 * p + 1
            Wp.append(np.concatenate([W[:, h0 * HD:(h0 + 1) * HD],
                                      W[:, h1 * HD:(h1 + 1) * HD]], axis=1) * SCL)
            bp[p, 0:64, 0] = bvec[h0 * HD:(h0 + 1) * HD] * SCL
            bp[p, 64:128, 0] = bvec[h1 * HD:(h1 + 1) * HD] * SCL
        return Wp, bp

    def pack_dr(Wp):
        # [128, 2048]: col = ((p*NCP+cp)*2 + j)*128 + m
        out = np.empty((128, 2048), f)
        for p in range(2):
            for cp in range(NCP):
                for j in range(2):
                    c = 2 * cp + j
                    blk = (p * NCP + cp) * 2 + j
                    out[:, blk * 128:(blk + 1) * 128] = \
                        Wp[p][c * 128:(c + 1) * 128, :]
        return out.astype(f8)

    def pack_plain(Wp):
        # [128, 2048]: col = p*1024 + c*128 + m
        return np.concatenate(
            [_to_sbuf_chunks(np.ascontiguousarray(Wp[p], f), NCH)
             for p in range(2)], axis=1).astype(bf)

    Wqp, bqp = pack_qk_pair(Wq, bq)
    Wkp, bkp = pack_qk_pair(Wk, bk)
    Wvl = np.ascontiguousarray(Wv[:, g * CLOC:(g + 1) * CLOC], f) * SCL
    wv_chunks = _to_sbuf_chunks(Wvl, NCH)
    Wol = np.ascontiguousarray(Wo[g * CLOC:(g + 1) * CLOC, :], f) * WOSCL
    wo_chunks = _to_sbuf_chunks(Wol, 2)

    return {"xt": xt, "xb": xb,
            "wq": pack_dr(Wqp), "wk": pack_dr(Wkp),
            "wqb": pack_plain(Wqp), "wkb": pack_plain(Wkp),
            "wv": wv_chunks.astype(f8), "wvb": wv_chunks.astype(bf),
            "wo": wo_chunks.astype(f8), "wob": wo_chunks.astype(bf),
            "bq": bqp, "bk": bkp,
            "ones": np.ones((128, NTT, NHL, 8), f8),
            "selw": _selw().astype(bf),
            "tri": np.triu(np.ones((128, 128))).astype(f8),
            "trib": np.triu(np.ones((128, 128))).astype(bf)}


def _run(inputs, trace=False, tmpdir=None):
    if "nc" not in _cache:
        _cache["nc"] = _build()
    nc = _cache["nc"]
    args = [np.asarray(inputs[k], np.float32) for k in
            ("x", "Wq", "bq", "Wk", "bk", "Wv", "bv", "Wo", "bo")]
    x, Wq, bq, Wk, bk, Wv, bv, Wo, bo = args
    in_maps = [_prep_core_inputs(c // 4, c % 4, x, Wq, bq, Wk, bk, Wv, bv, Wo, bo)
               for c in range(8)]
    res = bass_utils.run_bass_kernel_spmd(nc, in_maps, core_ids=list(range(8)),
                                          trace=trace, tmpdir=tmpdir)
    corr = (bv.astype(np.float64) @ Wo.astype(np.float64) + bo).astype(np.float64)
    out = np.empty((B, T, C), np.float32)
    for b in range(B):
        acc = np.zeros((T, C), np.float64)
        for g in range(4):
            acc += res.results[b * 4 + g]["out"].astype(np.float64)
        out[b] = (acc / OUT_DIV + corr).astype(np.float32)
    return out, res


def kernel(x, Wq, bq, Wk, bk, Wv, bv, Wo, bo):
    out, _ = _run(dict(x=x, Wq=Wq, bq=bq, Wk=Wk, bk=bk, Wv=Wv, bv=bv,
                       Wo=Wo, bo=bo))
    return out


def run_profiled(x, Wq, bq, Wk, bk, Wv, bv, Wo, bo, tmpdir=None):
    out, res = _run(dict(x=x, Wq=Wq, bq=bq, Wk=Wk, bk=bk, Wv=Wv, bv=bv,
                         Wo=Wo, bo=bo), trace=True, tmpdir=tmpdir)
    return out, res.exec_time_ns, res
